# revision 5
# baseline (speedup 1.0000x reference)
"""Trainium2 Bass kernel: per-tensor symmetric int8-quantized linear layer
(Brevitas-style), distributed over 8 NeuronCores.

    out = (round(x/sx) @ round(w/sw).T) * sx*sw + bias
    sx = max|x|/127 (global over x), sw = max|w|/127

Strategy (data-parallel over rows of x):
  - each core owns n/8 rows of x; weight/bias replicated
  - x is loaded once into SBUF (f32, resident), per-chunk absmax computed on
    VectorE while loading; per-partition maxes are AllGathered across the 8
    cores and reduced so every core has the exact global max|x|
  - int8 codes are stored as bf16 (ints <= 127 are exact in bf16); the int
    matmul accumulates exactly in f32 PSUM (|acc| <= 1024*127^2 < 2^24), so
    the quantized GEMM is bit-exact on the TensorEngine bf16 path
  - rounding uses the +1.5*2^23 trick == round-half-to-even (matches jnp.round)
  - quantized tiles are transposed with the XBAR dma transpose (bf16) to get
    the contraction dim onto partitions
  - epilogue: one fused VectorE op (psum * (sx*sw)) + bias, then DMA out
"""

import numpy as np

P = 128
N_TOTAL = 32768
K_DIM = 1024
M_DIM = 1024
N_CORES = 8
QMAX = 127.0
C_RNE = 12582912.0  # 1.5 * 2^23: forces f32 round-to-nearest-even to integer

_NC_CACHE = {}
_LAST_RESULTS = None


def build_nc(n_shard, k, m, n_cores):
    import concourse.mybir as mybir
    import concourse.tile as tile
    from concourse import bacc

    f32 = mybir.dt.float32
    bf16 = mybir.dt.bfloat16
    AX = mybir.AxisListType
    OP = mybir.AluOpType

    NT = n_shard // P   # n tiles per core
    KT = k // P         # contraction tiles
    MT = m // P         # weight row tiles
    XCH = 4 if NT % 4 == 0 else 1
    NCH = NT // XCH     # x load chunks
    NH = m // 512       # psum halves (moving free dim limit is 512)

    nc = bacc.Bacc("TRN2", target_bir_lowering=False, debug=False,
                   enable_asserts=False, num_devices=n_cores)
    x = nc.dram_tensor("x", [n_shard, k], f32, kind="ExternalInput").ap()
    w = nc.dram_tensor("weight", [m, k], f32, kind="ExternalInput").ap()
    b = nc.dram_tensor("bias", [m], f32, kind="ExternalInput").ap()
    out = nc.dram_tensor("out", [n_shard, m], f32, kind="ExternalOutput").ap()

    with tile.TileContext(nc) as tc:
        with (
            tc.tile_pool(name="res", bufs=1) as res,
            tc.tile_pool(name="wk", bufs=2) as wk,
            tc.tile_pool(name="psp", bufs=2, space="PSUM") as psp,
            tc.tile_pool(name="dram", bufs=1, space="DRAM") as dpool,
        ):
            x_sb = res.tile([P, NT, k], f32)
            qwT = res.tile([P, KT, m], bf16)
            bias_bc = res.tile([P, m], f32)
            xmax_acc = res.tile([P, NCH], f32)
            wmax_acc = res.tile([P, MT], f32)

            cc_in = dpool.tile([P], f32)
            cc_out = dpool.tile([P * n_cores], f32, addr_space="Shared")
            wscr = dpool.tile([P], f32)

            # bias broadcast to all partitions (tiny, off critical path)
            nc.gpsimd.dma_start(
                out=bias_bc[:],
                in_=b.rearrange("(o m) -> o m", o=1).broadcast_to([P, m]))

            # ---- x load (resident) + per-chunk absmax
            for c in range(NCH):
                nc.sync.dma_start(
                    out=x_sb[:, c * XCH:(c + 1) * XCH, :],
                    in_=x[c * XCH * P:(c + 1) * XCH * P, :]
                        .rearrange("(t p) k -> p t k", p=P))
                nc.vector.reduce_max(
                    xmax_acc[:, c:c + 1], x_sb[:, c * XCH:(c + 1) * XCH, :],
                    axis=AX.XY, apply_absolute_value=True)

            # ---- exact global max|x| via AllGather of per-partition maxes
            xmax_pp = res.tile([P, 1], f32)
            nc.vector.reduce_max(xmax_pp[:], xmax_acc[:], axis=AX.X,
                                 apply_absolute_value=False)
            nc.scalar.dma_start(out=cc_in[:], in_=xmax_pp[:])
            nc.gpsimd.collective_compute(
                "AllGather", OP.bypass,
                replica_groups=[list(range(n_cores))],
                ins=[cc_in[:].opt()], outs=[cc_out[:].opt()])
            xga = wk.tile([P, P * n_cores], f32, tag="xga", bufs=1)
            nc.scalar.dma_start(
                out=xga[:],
                in_=cc_out[:].rearrange("(o a) -> o a", o=1).broadcast_to([P, P * n_cores]))
            xmax_all = res.tile([P, 1], f32)
            nc.vector.reduce_max(xmax_all[:], xga[:], axis=AX.X,
                                 apply_absolute_value=False)
            sx = res.tile([P, 1], f32)
            rx = res.tile([P, 1], f32)
            nc.vector.tensor_scalar(sx[:], xmax_all[:], 1.0 / 127.0, None, OP.mult)
            nc.vector.reciprocal(rx[:], sx[:])

            # ---- weight absmax (w is replicated; local max == global max)
            for s_i in range(MT):
                wld = wk.tile([P, k], f32, tag="wld", bufs=2)
                nc.gpsimd.dma_start(out=wld[:], in_=w[s_i * P:(s_i + 1) * P, :])
                nc.vector.reduce_max(wmax_acc[:, s_i:s_i + 1], wld[:],
                                     axis=AX.X, apply_absolute_value=True)
            wmax_pp = res.tile([P, 1], f32)
            nc.vector.reduce_max(wmax_pp[:], wmax_acc[:], axis=AX.X,
                                 apply_absolute_value=False)
            nc.scalar.dma_start(out=wscr[:], in_=wmax_pp[:])
            wga = wk.tile([P, P], f32, tag="wga", bufs=1)
            nc.scalar.dma_start(
                out=wga[:],
                in_=wscr[:].rearrange("(o a) -> o a", o=1).broadcast_to([P, P]))
            wmax_all = res.tile([P, 1], f32)
            nc.vector.reduce_max(wmax_all[:], wga[:], axis=AX.X,
                                 apply_absolute_value=False)
            sw = res.tile([P, 1], f32)
            rw = res.tile([P, 1], f32)
            s_ap = res.tile([P, 1], f32)
            nc.vector.tensor_scalar(sw[:], wmax_all[:], 1.0 / 127.0, None, OP.mult)
            nc.vector.reciprocal(rw[:], sw[:])
            nc.vector.tensor_tensor(s_ap[:], sx[:], sw[:], OP.mult)

            # ---- quantize w (re-read from HBM during the collective window)
            for s_i in range(MT):
                wld2 = wk.tile([P, k], f32, tag="wld", bufs=2)
                nc.gpsimd.dma_start(out=wld2[:], in_=w[s_i * P:(s_i + 1) * P, :])
                wt1 = wk.tile([P, k], f32, tag="t1", bufs=2)
                nc.vector.tensor_scalar(wt1[:], wld2[:], rw[:], C_RNE,
                                        OP.mult, OP.add)
                qw_t = wk.tile([P, k], bf16, tag="q8", bufs=2)
                nc.vector.tensor_scalar(qw_t[:], wt1[:], C_RNE, None,
                                        OP.subtract)
                nc.scalar.dma_start(out=qwT[:, :, s_i * P:(s_i + 1) * P],
                                    in_=qw_t[:], transpose=True)

            # ---- main loop: quantize x tile, transpose, matmul, epilogue
            for i in range(NT):
                xt1 = wk.tile([P, k], f32, tag="t1", bufs=2)
                nc.vector.tensor_scalar(xt1[:], x_sb[:, i, :], rx[:], C_RNE,
                                        OP.mult, OP.add)
                qx_t = wk.tile([P, k], bf16, tag="q8", bufs=2)
                nc.vector.tensor_scalar(qx_t[:], xt1[:], C_RNE, None,
                                        OP.subtract)
                qxT = wk.tile([P, KT, P], bf16, tag="qxT", bufs=3)
                nc.scalar.dma_start(out=qxT[:], in_=qx_t[:], transpose=True)
                ps = psp.tile([P, m], f32)
                for t in range(KT):
                    for h in range(NH):
                        nc.tensor.matmul(
                            ps[:, h * 512:(h + 1) * 512],
                            qxT[:, t, :],
                            qwT[:, t, h * 512:(h + 1) * 512],
                            start=(t == 0), stop=(t == KT - 1))
                out_t = wk.tile([P, m], f32, tag="out_t", bufs=2)
                nc.vector.scalar_tensor_tensor(
                    out_t[:], ps[:], s_ap[:], bias_bc[:], OP.mult, OP.add)
                nc.sync.dma_start(out=out[i * P:(i + 1) * P, :], in_=out_t[:])

    nc.compile()
    return nc


def _get_nc(n_shard, k, m, n_cores):
    key = (n_shard, k, m, n_cores)
    if key not in _NC_CACHE:
        _NC_CACHE[key] = build_nc(n_shard, k, m, n_cores)
    return _NC_CACHE[key]


def kernel(x, weight, bias):
    x = np.ascontiguousarray(np.asarray(x, dtype=np.float32))
    weight = np.ascontiguousarray(np.asarray(weight, dtype=np.float32))
    bias = np.ascontiguousarray(np.asarray(bias, dtype=np.float32))
    n, k = x.shape
    m = weight.shape[0]
    n_cores = N_CORES
    shard = n // n_cores

    from concourse.bass_utils import run_bass_kernel_spmd
    nc = _get_nc(shard, k, m, n_cores)
    in_maps = [
        {"x": np.ascontiguousarray(x[c * shard:(c + 1) * shard]),
         "weight": weight, "bias": bias}
        for c in range(n_cores)
    ]
    res = run_bass_kernel_spmd(nc, in_maps, core_ids=list(range(n_cores)))
    global _LAST_RESULTS
    _LAST_RESULTS = res
    return np.concatenate([r["out"] for r in res.results], axis=0)
